# revision 5
# baseline (speedup 1.0000x reference)
"""Causal self-attention (B=4, T=2048, D=1024, H=16, DH=64) on 8 TRN2 NeuronCores.

Sharding: core c handles batch b = c//2 and head group hg = (c%2)*8 (8 of 16
heads), Megatron-style on the head dim. Each core computes QKV for its heads,
causal attention, and its partial output projection; the host sums the two
partial projections per batch.

On-chip layout (per core):
  - qkv computed transposed: q^T/k^T as [feat(128-part), tok] tiles, v in
    natural [tok, feat] layout with an appended ones column so the PV matmul
    also produces the softmax normalizer l.
  - softmax without max-subtraction (scores ~ N(0,1): exp never overflows);
    causal masking by multiplying exp tiles with 0/1 masks on diagonal blocks,
    fully-masked blocks are skipped.
  - all matmuls in float32r (full PE rate at N=512, ~tf32 precision).
"""
import sys
import types

import numpy as np

# If the image lacks antenv.axon_hooks, register a compatible stub so
# run_bass_kernel_spmd(trace=True)/BASS_TRACE=1 can capture NTFF profiles
# (falls back to no-op when the axon client library has no profile export).
try:
    import antenv.axon_hooks  # noqa: F401
except ImportError:
    try:
        from trn_agent_boot.trn_boot import _ntff_profile_via_ctypes

        _hook = _ntff_profile_via_ctypes("/opt/axon/libaxon_pjrt.so")
    except Exception:
        _hook = None
    _m = types.ModuleType("antenv.axon_hooks")
    _m.get_axon_ntff_profile_hook = lambda: _hook
    _m.set_axon_ntff_profile_hook = lambda h: None
    sys.modules["antenv.axon_hooks"] = _m

import concourse.bass_utils as _bass_utils

if getattr(_bass_utils, "_local_artifacts_patch", None) is None:
    _bass_utils.upload_artifacts = lambda tmpdir: tmpdir
    _bass_utils._local_artifacts_patch = True

import concourse.bacc as bacc
import concourse.tile as tile
from concourse import mybir
from concourse.bass_utils import run_bass_kernel_spmd

F32 = mybir.dt.float32
F32R = mybir.dt.float32r
EXP = mybir.ActivationFunctionType.Exp

B, T, D = 4, 2048, 1024
H, DH = 16, 64
HPC = 8             # heads per core
P = 128
NSLAB = T // 512    # 4 query slabs
DC = D // P         # 8 d-chunks
N_CORES = 8

_cached_nc = None
LAST_EXEC_NS = None


def _build_program():
    nc = bacc.Bacc("TRN2", target_bir_lowering=False, debug=False, num_devices=N_CORES)
    xt_d = nc.dram_tensor("xt", [D, T], F32R, kind="ExternalInput").ap()
    wqk_d = nc.dram_tensor("wqk", [D, 2 * HPC * DH], F32R, kind="ExternalInput").ap()
    wv_d = nc.dram_tensor("wv", [D, HPC * DH], F32R, kind="ExternalInput").ap()
    wp_d = nc.dram_tensor("wp", [HPC * DH, D], F32R, kind="ExternalInput").ap()
    masks_d = nc.dram_tensor("masks", [P, 384], F32R, kind="ExternalInput").ap()
    out_d = nc.dram_tensor("out", [T, D], F32, kind="ExternalOutput").ap()

    with tile.TileContext(nc) as tc:
        lp = nc.allow_low_precision(reason="fp32r matmul inputs")
        lp.__enter__()
        with (
            tc.tile_pool(name="persist", bufs=1) as persist,
            tc.tile_pool(name="small", bufs=1) as small,
        ):
            # masks[:, 0:128] = within-tile triangle (q_local >= k_local);
            # masks[:, 128:384] = p=3 tail: cols 256..512 of the slab (zeros then triangle)
            masks = persist.tile([P, 384], F32R)
            nc.sync.dma_start(masks, masks_d)
            # q^T/k^T feature tiles: f in 0..3 -> q feats 128f..;  4..7 -> k
            qk = persist.tile([P, 8, T], F32R)
            # v natural layout + ones column: [tok-tile, head, dh+1]
            vt = persist.tile([P, T // P, HPC, DH + 1], F32R)
            ones_f = small.tile([P, (T // P) * HPC], F32)
            nc.vector.memset(ones_f, 1.0)
            nc.vector.tensor_copy(
                vt[:, :, :, DH : DH + 1],
                ones_f.rearrange("p (a b) -> p a b", a=T // P).unsqueeze(3),
            )
            r_all = small.tile([P, 512], F32)

            # ---- Phase 1: QKV projections ----
            with (
                tc.tile_pool(name="xs", bufs=2) as xpool,
                tc.tile_pool(name="w1", bufs=1) as wpool,
                tc.tile_pool(name="qkps", bufs=3, space="PSUM") as qkps,
                tc.tile_pool(name="vps", bufs=2, space="PSUM") as vps,
            ):
                wqk = wpool.tile([P, DC, 2 * HPC * DH], F32R)
                wv = wpool.tile([P, DC, HPC * DH], F32R)
                for c in range(DC):
                    nc.sync.dma_start(wqk[:, c, :], wqk_d[P * c : P * (c + 1), :])
                    nc.sync.dma_start(wv[:, c, :], wv_d[P * c : P * (c + 1), :])
                for j in range(NSLAB):
                    xs = xpool.tile([P, DC, 512], F32R, tag="xs")
                    for c in range(DC):
                        nc.sync.dma_start(
                            xs[:, c, :], xt_d[P * c : P * (c + 1), 512 * j : 512 * (j + 1)]
                        )
                    for f in range(8):
                        ps = qkps.tile([P, 512], F32, tag="qk")
                        for c in range(DC):
                            nc.tensor.matmul(
                                ps,
                                wqk[:, c, P * f : P * (f + 1)],
                                xs[:, c, :],
                                start=(c == 0),
                                stop=(c == DC - 1),
                            )
                        nc.vector.tensor_copy(qk[:, f, 512 * j : 512 * (j + 1)], ps)
                    for tt in range(4):
                        psv = vps.tile([P, 512], F32, tag="v")
                        for c in range(DC):
                            nc.tensor.matmul(
                                psv,
                                xs[:, c, P * tt : P * (tt + 1)],
                                wv[:, c, :],
                                start=(c == 0),
                                stop=(c == DC - 1),
                            )
                        nc.vector.tensor_copy(
                            vt[:, 4 * j + tt, :, 0:DH],
                            psv.rearrange("p (h d) -> p h d", h=HPC),
                        )

            # ---- Phase 2: attention + output projection, per query slab ----
            with (
                tc.tile_pool(name="yt", bufs=1) as ytpool,
                tc.tile_pool(name="w2", bufs=1) as w2pool,
                tc.tile_pool(name="pp", bufs=4) as ppool,
                tc.tile_pool(name="tails", bufs=2) as tails,
                tc.tile_pool(name="outsb", bufs=3) as outsb,
                tc.tile_pool(name="sps", bufs=4, space="PSUM") as sps,
                tc.tile_pool(name="pvps", bufs=2, space="PSUM") as pvps,
                tc.tile_pool(name="projps", bufs=2, space="PSUM") as projps,
            ):
                # y^T packed: chunk c rows 0..63 head 2c, 64..127 head 2c+1
                yt = ytpool.tile([P, HPC // 2, T], F32R)
                wp = w2pool.tile([P, HPC * DH // P, D], F32R)
                for c in range(HPC * DH // P):
                    nc.sync.dma_start(wp[:, c, :], wp_d[P * c : P * (c + 1), :])
                def emit_proj(j):
                    for tt in range(4 * j, 4 * j + 4):
                        for e in range(2):
                            pp = projps.tile([P, 512], F32, tag="pj")
                            for c in range(HPC * DH // P):
                                nc.tensor.matmul(
                                    pp,
                                    yt[:, c, P * tt : P * (tt + 1)],
                                    wp[:, c, 512 * e : 512 * (e + 1)],
                                    start=(c == 0),
                                    stop=(c == HPC * DH // P - 1),
                                )
                            ob = outsb.tile([P, 512], F32, tag="ob")
                            nc.vector.tensor_copy(ob, pp)
                            nc.sync.dma_start(
                                out_d[P * tt : P * (tt + 1), 512 * e : 512 * (e + 1)], ob
                            )

                # per diagonal position p: column offset the tile is computed from
                C0 = (0, 128, 256, 256)
                for j in range(NSLAB):
                    kmax = 4 * j + 4
                    for h in range(HPC):
                        hoff = (h % 2) * 64
                        qf, kf = h // 2, 4 + h // 2
                        pv = pvps.tile([P, 512], F32, tag="pv")

                        def c0_of(i):
                            return C0[i - 4 * j] if i >= 4 * j else 0

                        p_tiles = {}

                        def emit_s(i):
                            c0 = c0_of(i)
                            s_ps = sps.tile([P, 512], F32, tag="s")
                            nc.tensor.matmul(
                                s_ps[:, c0:512],
                                qk[hoff : hoff + 64, kf, P * i : P * (i + 1)],
                                qk[hoff : hoff + 64, qf, 512 * j + c0 : 512 * (j + 1)],
                                start=True,
                                stop=True,
                            )
                            p_sb = ppool.tile([P, 512], F32R, tag="p")
                            nc.scalar.activation(
                                p_sb[:, c0:512], s_ps[:, c0:512], EXP, scale=1.0 / 8.0
                            )
                            if i >= 4 * j:
                                p = i - 4 * j
                                if p == 3:
                                    nc.vector.tensor_mul(
                                        p_sb[:, 256:512], p_sb[:, 256:512], masks[:, 128:384]
                                    )
                                else:
                                    nc.vector.tensor_mul(
                                        p_sb[:, P * p : P * (p + 1)],
                                        p_sb[:, P * p : P * (p + 1)],
                                        masks[:, 0:128],
                                    )
                            p_tiles[i] = p_sb

                        def emit_pv(i):
                            c0 = c0_of(i)
                            nc.tensor.matmul(
                                pv[0:65, c0:512],
                                vt[:, i, h, :],
                                p_tiles.pop(i)[:, c0:512],
                                start=(i == 0),
                                stop=(i == kmax - 1),
                            )

                        # skew-2 software pipeline: s(0),s(1),s(2),pv(0),s(3),pv(1),...
                        SKEW = 2
                        for i in range(kmax + SKEW):
                            if i < kmax:
                                emit_s(i)
                            if i >= SKEW:
                                emit_pv(i - SKEW)
                        nc.vector.reciprocal(r_all[64:65, :], pv[64:65, :])
                        rb = tails.tile([64, 512], F32, tag="rb")
                        r0 = tails.tile([1, 512], F32, tag="r0")
                        nc.sync.dma_start(r0, r_all[64:65, :])
                        nc.gpsimd.partition_broadcast(rb, r0, channels=64)
                        if h % 2 == 0:
                            nc.vector.tensor_mul(
                                yt[0:64, qf, 512 * j : 512 * (j + 1)], pv[0:64, :], rb
                            )
                        else:
                            ytmp = tails.tile([64, 512], F32R, tag="ytmp")
                            nc.vector.tensor_mul(ytmp, pv[0:64, :], rb)
                            nc.sync.dma_start(yt[64:128, qf, 512 * j : 512 * (j + 1)], ytmp)
                    if j > 0:
                        emit_proj(j - 1)
                emit_proj(NSLAB - 1)
        lp.__exit__(None, None, None)
    nc.compile()
    return nc


def _host_masks():
    m = np.zeros((P, 384), dtype=np.float32)
    for kl in range(P):
        m[kl, 0:128] = (np.arange(128) >= kl).astype(np.float32)       # triangle
        m[kl, 128:384] = (np.arange(256, 512) >= kl + 384).astype(np.float32)  # p=3 tail
    return m


def kernel(x, w_attn, w_proj):
    global _cached_nc, LAST_EXEC_NS
    x = np.asarray(x, dtype=np.float32)
    w_attn = np.asarray(w_attn, dtype=np.float32)
    w_proj = np.asarray(w_proj, dtype=np.float32)

    if _cached_nc is None:
        _cached_nc = _build_program()
    nc = _cached_nc

    masks = _host_masks()
    in_maps = []
    for c in range(N_CORES):
        b, hg = c // 2, (c % 2) * HPC
        w_q = w_attn[hg * DH : hg * DH + HPC * DH, :]
        w_k = w_attn[D + hg * DH : D + hg * DH + HPC * DH, :]
        w_v = w_attn[2 * D + hg * DH : 2 * D + hg * DH + HPC * DH, :]
        in_maps.append(
            {
                "xt": np.ascontiguousarray(x[b].T),
                "wqk": np.ascontiguousarray(np.concatenate([w_q, w_k], axis=0).T),
                "wv": np.ascontiguousarray(w_v.T),
                "wp": np.ascontiguousarray(w_proj[:, hg * DH : hg * DH + HPC * DH].T),
                "masks": masks,
            }
        )

    res = run_bass_kernel_spmd(nc, in_maps, list(range(N_CORES)))
    LAST_EXEC_NS = res.exec_time_ns
    y = np.empty((B, T, D), dtype=np.float32)
    for b in range(B):
        y[b] = res.results[2 * b]["out"] + res.results[2 * b + 1]["out"]
    return y


# revision 7
# speedup vs baseline: 1.1281x; 1.1281x over previous
"""Causal self-attention (B=4, T=2048, D=1024, H=16, DH=64) on 8 TRN2 NeuronCores.

Sharding: core c handles batch b = c//2 and head group hg = (c%2)*8 (8 of 16
heads), Megatron-style on the head dim. Each core computes QKV for its heads,
causal attention, and its partial output projection; the host sums the two
partial projections per batch.

On-chip layout (per core):
  - qkv computed transposed: q^T/k^T as [feat(128-part), tok] tiles, v in
    natural [tok, feat] layout with an appended ones column so the PV matmul
    also produces the softmax normalizer l.
  - softmax without max-subtraction (scores ~ N(0,1): exp never overflows);
    causal masking by multiplying exp tiles with 0/1 masks on diagonal blocks,
    fully-masked blocks are skipped.
  - all matmuls in float32r (full PE rate at N=512, ~tf32 precision).
"""
import sys
import types

import numpy as np

# If the image lacks antenv.axon_hooks, register a compatible stub so
# run_bass_kernel_spmd(trace=True)/BASS_TRACE=1 can capture NTFF profiles
# (falls back to no-op when the axon client library has no profile export).
try:
    import antenv.axon_hooks  # noqa: F401
except ImportError:
    try:
        from trn_agent_boot.trn_boot import _ntff_profile_via_ctypes

        _hook = _ntff_profile_via_ctypes("/opt/axon/libaxon_pjrt.so")
    except Exception:
        _hook = None
    _m = types.ModuleType("antenv.axon_hooks")
    _m.get_axon_ntff_profile_hook = lambda: _hook
    _m.set_axon_ntff_profile_hook = lambda h: None
    sys.modules["antenv.axon_hooks"] = _m

import concourse.bass_utils as _bass_utils

if getattr(_bass_utils, "_local_artifacts_patch", None) is None:
    _bass_utils.upload_artifacts = lambda tmpdir: tmpdir
    _bass_utils._local_artifacts_patch = True

import concourse.bacc as bacc
import concourse.tile as tile
from concourse import mybir
from concourse.bass_utils import run_bass_kernel_spmd

F32 = mybir.dt.float32
F32R = mybir.dt.float32r
EXP = mybir.ActivationFunctionType.Exp

B, T, D = 4, 2048, 1024
H, DH = 16, 64
HPC = 8             # heads per core
P = 128
NSLAB = T // 512    # 4 query slabs
DC = D // P         # 8 d-chunks
N_CORES = 8

_cached_nc = None
LAST_EXEC_NS = None


def _build_program():
    nc = bacc.Bacc("TRN2", target_bir_lowering=False, debug=False, num_devices=N_CORES)
    xt_d = nc.dram_tensor("xt", [D, T], F32R, kind="ExternalInput").ap()
    wqk_d = nc.dram_tensor("wqk", [D, 2 * HPC * DH], F32R, kind="ExternalInput").ap()
    wv_d = nc.dram_tensor("wv", [D, HPC * DH], F32R, kind="ExternalInput").ap()
    wp_d = nc.dram_tensor("wp", [HPC * DH, D], F32R, kind="ExternalInput").ap()
    masks_d = nc.dram_tensor("masks", [P, 384], F32R, kind="ExternalInput").ap()
    out_d = nc.dram_tensor("out", [T, D], F32, kind="ExternalOutput").ap()

    # DRAM views with the 128-partition chunk dim split out
    wqk_v = wqk_d.rearrange("(c p) f -> p c f", p=P)
    wv_v = wv_d.rearrange("(c p) f -> p c f", p=P)
    wp_v = wp_d.rearrange("(c p) f -> p c f", p=P)
    xt_v = xt_d.rearrange("(c p) t -> p c t", p=P)

    with tile.TileContext(nc) as tc:
        lp = nc.allow_low_precision(reason="fp32r matmul inputs")
        lp.__enter__()
        with (
            tc.tile_pool(name="persist", bufs=1) as persist,
            tc.tile_pool(name="small", bufs=1) as small,
            tc.tile_pool(name="xs", bufs=1) as xpool,
            tc.tile_pool(name="wqkf", bufs=2) as wqkfpool,
            tc.tile_pool(name="wvs", bufs=1) as wvpool,
            tc.tile_pool(name="wp2", bufs=1) as wppool,
            tc.tile_pool(name="yt", bufs=1) as ytpool,
            tc.tile_pool(name="pp", bufs=4) as ppool,
            tc.tile_pool(name="tails", bufs=2) as tails,
            tc.tile_pool(name="outsb", bufs=2) as outsb,
            tc.tile_pool(name="qkps", bufs=2, space="PSUM") as qkps,
            tc.tile_pool(name="sps", bufs=4, space="PSUM") as sps,
            tc.tile_pool(name="pvps", bufs=2, space="PSUM") as pvps,
        ):
            # masks[:, 0:128] = within-tile triangle (q_local >= k_local);
            # masks[:, 128:384] = p=3 tail: slab cols 256..512 (zeros then triangle)
            masks = persist.tile([P, 384], F32R)
            nc.sync.dma_start(masks, masks_d)
            # k^T persistent feature tiles; q^T lives in a 2-slab ring (a slab's
            # q is only read by its own attention pass)
            qk_k = persist.tile([P, 4, T], F32R)
            qk_q = persist.tile([P, 4, 2, 512], F32R)
            # v natural layout + ones column: [tok-tile, head, dh+1]
            vt = persist.tile([P, T // P, HPC, DH + 1], F32R)
            ones_f = small.tile([P, (T // P) * HPC], F32)
            nc.vector.memset(ones_f, 1.0)
            nc.vector.tensor_copy(
                vt[:, :, :, DH : DH + 1],
                ones_f.rearrange("p (a b) -> p a b", a=T // P).unsqueeze(3),
            )
            r_all = small.tile([P, 512], F32)
            lg = small.tile([P, 512], F32)       # per-slab l rows (partitions 0..7); recip in place
            # y^T ring: slab j uses ring j%2; chunk c rows 0..63 head 2c, 64..127 head 2c+1
            yt = ytpool.tile([P, HPC // 2, 2, 512], F32R)
            wp = wppool.tile([P, HPC * DH // P, D], F32R)
            for c in range(HPC * DH // P):
                nc.sync.dma_start(wp[:, c, :], wp_v[:, c, :])

            wv_s = wvpool.tile([P, DC, 512], F32R)
            nc.sync.dma_start(wv_s, wv_v)

            def emit_qkv_chunks(j):
                """Returns a list of emitter thunks for slab j's QKV work."""
                xs = xpool.tile([P, DC, 512], F32R, tag="xs")

                def load():
                    for c in range(DC):
                        nc.sync.dma_start(xs[:, c, :], xt_v[:, c, 512 * j : 512 * (j + 1)])

                def f_chain(f):
                    wqk_f = wqkfpool.tile([P, DC, P], F32R, tag="wqkf")
                    nc.sync.dma_start(wqk_f, wqk_v[:, :, P * f : P * (f + 1)])
                    ps = qkps.tile([P, 512], F32, tag="qk")
                    for c in range(DC):
                        nc.tensor.matmul(
                            ps, wqk_f[:, c, :], xs[:, c, :],
                            start=(c == 0), stop=(c == DC - 1),
                        )
                    if f < 4:
                        nc.vector.tensor_copy(qk_q[:, f, j % 2, :], ps)
                    else:
                        nc.vector.tensor_copy(qk_k[:, f - 4, 512 * j : 512 * (j + 1)], ps)

                def v_chain(tt):
                    psv = qkps.tile([P, 512], F32, tag="qk")
                    for c in range(DC):
                        nc.tensor.matmul(
                            psv, xs[:, c, P * tt : P * (tt + 1)], wv_s[:, c, :],
                            start=(c == 0), stop=(c == DC - 1),
                        )
                    nc.vector.tensor_copy(
                        vt[:, 4 * j + tt, :, 0:DH],
                        psv.rearrange("p (h d) -> p h d", h=HPC),
                    )

                thunks = [load]
                thunks += [(lambda f=f: f_chain(f)) for f in range(8)]
                thunks += [(lambda tt=tt: v_chain(tt)) for tt in range(4)]
                return thunks

            def emit_proj(j):
                r = j % 2
                for lt in range(4):
                    tt = 4 * j + lt
                    for e in range(2):
                        pp = sps.tile([P, 512], F32, tag="s")
                        for c in range(HPC * DH // P):
                            nc.tensor.matmul(
                                pp,
                                yt[:, c, r, P * lt : P * (lt + 1)],
                                wp[:, c, 512 * e : 512 * (e + 1)],
                                start=(c == 0),
                                stop=(c == HPC * DH // P - 1),
                            )
                        ob = outsb.tile([P, 512], F32, tag="ob")
                        nc.vector.tensor_copy(ob, pp)
                        nc.sync.dma_start(
                            out_d[P * tt : P * (tt + 1), 512 * e : 512 * (e + 1)], ob
                        )

            # per diagonal position p: column offset the tile is computed from
            C0 = (0, 128, 256, 256)

            def attn_head(j, h):
                r = j % 2
                kmax = 4 * j + 4
                hoff = (h % 2) * 64
                qf, kf = h // 2, h // 2
                pv = pvps.tile([P, 512], F32, tag="pv")

                def c0_of(i):
                    return C0[i - 4 * j] if i >= 4 * j else 0

                p_tiles = {}

                def emit_s(i):
                    c0 = c0_of(i)
                    s_ps = sps.tile([P, 512], F32, tag="s")
                    nc.tensor.matmul(
                        s_ps[:, c0:512],
                        qk_k[hoff : hoff + 64, kf, P * i : P * (i + 1)],
                        qk_q[hoff : hoff + 64, qf, r, c0:512],
                        start=True,
                        stop=True,
                    )
                    p_sb = ppool.tile([P, 512], F32R, tag="p")
                    nc.scalar.activation(p_sb[:, c0:512], s_ps[:, c0:512], EXP, scale=1.0 / 8.0)
                    if i >= 4 * j:
                        p = i - 4 * j
                        if p == 3:
                            nc.vector.tensor_mul(
                                p_sb[:, 256:512], p_sb[:, 256:512], masks[:, 128:384]
                            )
                        else:
                            nc.vector.tensor_mul(
                                p_sb[:, P * p : P * (p + 1)],
                                p_sb[:, P * p : P * (p + 1)],
                                masks[:, 0:128],
                            )
                    p_tiles[i] = p_sb

                def emit_pv(i):
                    c0 = c0_of(i)
                    nc.tensor.matmul(
                        pv[0:65, c0:512],
                        vt[:, i, h, :],
                        p_tiles.pop(i)[:, c0:512],
                        start=(i == 0),
                        stop=(i == kmax - 1),
                    )

                SKEW = 3
                for i in range(kmax + SKEW):
                    if i < kmax:
                        emit_s(i)
                    if i >= SKEW:
                        emit_pv(i - SKEW)
                # stash l and unnormalized y^T; batched reciprocal at slab end
                nc.vector.tensor_copy(r_all[64:65, :], pv[64:65, :])
                nc.sync.dma_start(lg[h : h + 1, :], r_all[64:65, :])
                if h % 2 == 0:
                    nc.vector.tensor_copy(yt[0:64, qf, r, :], pv[0:64, :])
                else:
                    ytmp = tails.tile([64, 512], F32R, tag="ytmp")
                    nc.vector.tensor_copy(ytmp, pv[0:64, :])
                    nc.sync.dma_start(yt[64:128, qf, r, :], ytmp)

            def slab_tail(j):
                r = j % 2
                nc.vector.reciprocal(lg[0:HPC, :], lg[0:HPC, :])
                for h in range(HPC):
                    r0 = tails.tile([1, 512], F32, tag="r0")
                    nc.sync.dma_start(r0, lg[h : h + 1, :])
                    rb = tails.tile([P, 512], F32, tag="rb")
                    nc.gpsimd.partition_broadcast(rb, r0, channels=P)
                    qf = h // 2
                    if h % 2 == 0:
                        nc.vector.tensor_mul(yt[0:64, qf, r, :], yt[0:64, qf, r, :], rb[0:64, :])
                    else:
                        nc.vector.tensor_mul(
                            yt[64:128, qf, r, :], yt[64:128, qf, r, :], rb[64:128, :]
                        )

            # ---- pipelined emission ----
            for th in emit_qkv_chunks(0):
                th()
            pending = emit_qkv_chunks(1)
            for j in range(NSLAB):
                for h in range(HPC):
                    attn_head(j, h)
                    # interleave next slab's QKV chunks across this slab's heads
                    want = (len(pending) * (h + 1) + HPC - 1) // HPC if pending else 0
                    done = (len(pending) * h + HPC - 1) // HPC if pending and h else 0
                    for th in pending[done:want]:
                        th()
                slab_tail(j)
                if j + 2 < NSLAB:
                    pending = emit_qkv_chunks(j + 2)
                else:
                    pending = []
                if j > 0:
                    emit_proj(j - 1)
            emit_proj(NSLAB - 1)
        lp.__exit__(None, None, None)
    nc.compile()
    return nc


def _host_masks():
    m = np.zeros((P, 384), dtype=np.float32)
    for kl in range(P):
        m[kl, 0:128] = (np.arange(128) >= kl).astype(np.float32)       # triangle
        m[kl, 128:384] = (np.arange(256, 512) >= kl + 384).astype(np.float32)  # p=3 tail
    return m


def kernel(x, w_attn, w_proj):
    global _cached_nc, LAST_EXEC_NS
    x = np.asarray(x, dtype=np.float32)
    w_attn = np.asarray(w_attn, dtype=np.float32)
    w_proj = np.asarray(w_proj, dtype=np.float32)

    if _cached_nc is None:
        _cached_nc = _build_program()
    nc = _cached_nc

    masks = _host_masks()
    in_maps = []
    for c in range(N_CORES):
        b, hg = c // 2, (c % 2) * HPC
        w_q = w_attn[hg * DH : hg * DH + HPC * DH, :]
        w_k = w_attn[D + hg * DH : D + hg * DH + HPC * DH, :]
        w_v = w_attn[2 * D + hg * DH : 2 * D + hg * DH + HPC * DH, :]
        in_maps.append(
            {
                "xt": np.ascontiguousarray(x[b].T),
                "wqk": np.ascontiguousarray(np.concatenate([w_q, w_k], axis=0).T),
                "wv": np.ascontiguousarray(w_v.T),
                "wp": np.ascontiguousarray(w_proj[:, hg * DH : hg * DH + HPC * DH].T),
                "masks": masks,
            }
        )

    res = run_bass_kernel_spmd(nc, in_maps, list(range(N_CORES)))
    LAST_EXEC_NS = res.exec_time_ns
    y = np.empty((B, T, D), dtype=np.float32)
    for b in range(B):
        y[b] = res.results[2 * b]["out"] + res.results[2 * b + 1]["out"]
    return y


# revision 8
# speedup vs baseline: 1.3205x; 1.1705x over previous
"""Causal self-attention (B=4, T=2048, D=1024, H=16, DH=64) on 8 TRN2 NeuronCores.

Sharding: core c handles batch b = c//2 and head group hg = (c%2)*8 (8 of 16
heads), Megatron-style on the head dim. Each core computes QKV for its heads,
causal attention, and its partial output projection; the host sums the two
partial projections per batch.

On-chip layout (per core):
  - qkv computed transposed: q^T/k^T as [feat(128-part), tok] tiles, v in
    natural [tok, feat] layout with an appended ones column so the PV matmul
    also produces the softmax normalizer l.
  - softmax without max-subtraction (scores ~ N(0,1): exp never overflows);
    causal masking by multiplying exp tiles with 0/1 masks on diagonal blocks,
    fully-masked blocks are skipped.
  - all matmuls in float32r (full PE rate at N=512, ~tf32 precision).
"""
import sys
import types

import numpy as np

# If the image lacks antenv.axon_hooks, register a compatible stub so
# run_bass_kernel_spmd(trace=True)/BASS_TRACE=1 can capture NTFF profiles
# (falls back to no-op when the axon client library has no profile export).
try:
    import antenv.axon_hooks  # noqa: F401
except ImportError:
    try:
        from trn_agent_boot.trn_boot import _ntff_profile_via_ctypes

        _hook = _ntff_profile_via_ctypes("/opt/axon/libaxon_pjrt.so")
    except Exception:
        _hook = None
    _m = types.ModuleType("antenv.axon_hooks")
    _m.get_axon_ntff_profile_hook = lambda: _hook
    _m.set_axon_ntff_profile_hook = lambda h: None
    sys.modules["antenv.axon_hooks"] = _m

import concourse.bass_utils as _bass_utils

if getattr(_bass_utils, "_local_artifacts_patch", None) is None:
    _bass_utils.upload_artifacts = lambda tmpdir: tmpdir
    _bass_utils._local_artifacts_patch = True

import concourse.bacc as bacc
import concourse.tile as tile
from concourse import mybir
from concourse.bass_utils import run_bass_kernel_spmd

F32 = mybir.dt.float32
F32R = mybir.dt.float32r
EXP = mybir.ActivationFunctionType.Exp

B, T, D = 4, 2048, 1024
H, DH = 16, 64
HPC = 8             # heads per core
P = 128
NSLAB = T // 512    # 4 query slabs
DC = D // P         # 8 d-chunks
N_CORES = 8

_cached_nc = None
LAST_EXEC_NS = None


def _build_program():
    nc = bacc.Bacc("TRN2", target_bir_lowering=False, debug=False, num_devices=N_CORES)
    xt_d = nc.dram_tensor("xt", [D, T], F32R, kind="ExternalInput").ap()
    wqk_d = nc.dram_tensor("wqk", [D, 2 * HPC * DH], F32R, kind="ExternalInput").ap()
    wv_d = nc.dram_tensor("wv", [D, HPC * DH], F32R, kind="ExternalInput").ap()
    wp_d = nc.dram_tensor("wp", [HPC * DH, D], F32R, kind="ExternalInput").ap()
    masks_d = nc.dram_tensor("masks", [P, 384], F32R, kind="ExternalInput").ap()
    out_d = nc.dram_tensor("out", [T, D], F32, kind="ExternalOutput").ap()

    # DRAM views with the 128-partition chunk dim split out
    wqk_v = wqk_d.rearrange("(c p) f -> p c f", p=P)
    wv_v = wv_d.rearrange("(c p) f -> p c f", p=P)
    wp_v = wp_d.rearrange("(c p) f -> p c f", p=P)
    xt_v = xt_d.rearrange("(c p) t -> p c t", p=P)

    with tile.TileContext(nc) as tc:
        lp = nc.allow_low_precision(reason="fp32r matmul inputs")
        lp.__enter__()
        with (
            tc.tile_pool(name="persist", bufs=1) as persist,
            tc.tile_pool(name="small", bufs=1) as small,
            tc.tile_pool(name="xs", bufs=1) as xpool,
            tc.tile_pool(name="wqkf", bufs=2) as wqkfpool,
            tc.tile_pool(name="wvs", bufs=1) as wvpool,
            tc.tile_pool(name="wp2", bufs=1) as wppool,
            tc.tile_pool(name="yt", bufs=1) as ytpool,
            tc.tile_pool(name="pp", bufs=4) as ppool,
            tc.tile_pool(name="tails", bufs=2) as tails,
            tc.tile_pool(name="outsb", bufs=2) as outsb,
            tc.tile_pool(name="qkps", bufs=2, space="PSUM") as qkps,
            tc.tile_pool(name="sps", bufs=4, space="PSUM") as sps,
            tc.tile_pool(name="pvps", bufs=2, space="PSUM") as pvps,
        ):
            # masks[:, 0:128] = within-tile triangle (q_local >= k_local);
            # masks[:, 128:384] = p=3 tail: slab cols 256..512 (zeros then triangle)
            masks = persist.tile([P, 384], F32R)
            nc.sync.dma_start(masks, masks_d)
            # k^T persistent feature tiles; q^T lives in a 2-slab ring (a slab's
            # q is only read by its own attention pass)
            qk_k = persist.tile([P, 4, T], F32R)
            qk_q = persist.tile([P, 4, 2, 512], F32R)
            # v natural layout + ones column: [tok-tile, head, dh+1]
            vt = persist.tile([P, T // P, HPC, DH + 1], F32R)
            ones_f = small.tile([P, (T // P) * HPC], F32)
            nc.vector.memset(ones_f, 1.0)
            nc.vector.tensor_copy(
                vt[:, :, :, DH : DH + 1],
                ones_f.rearrange("p (a b) -> p a b", a=T // P).unsqueeze(3),
            )
            r_all = small.tile([P, 512], F32)
            lg = small.tile([P, 512], F32)       # per-slab l rows (partitions 0..7); recip in place
            # y^T ring: slab j uses ring j%2; chunk c rows 0..63 head 2c, 64..127 head 2c+1
            yt = ytpool.tile([P, HPC // 2, 2, 512], F32R)
            wp = wppool.tile([P, HPC * DH // P, D], F32R)
            for c in range(HPC * DH // P):
                nc.sync.dma_start(wp[:, c, :], wp_v[:, c, :])

            wv_s = wvpool.tile([P, DC, 512], F32R)
            nc.sync.dma_start(wv_s, wv_v)

            def filler_gen(j):
                """Generator emitting one PE filler matmul per next(): next slab's
                QKV chains and the previous slab's projection chains."""
                if j + 1 < NSLAB:
                    jn = j + 1
                    xs = xpool.tile([P, DC, 512], F32R, tag="xs")
                    for c in range(DC):
                        nc.sync.dma_start(xs[:, c, :], xt_v[:, c, 512 * jn : 512 * (jn + 1)])
                    for f in range(8):
                        wqk_f = wqkfpool.tile([P, DC, P], F32R, tag="wqkf")
                        nc.sync.dma_start(wqk_f, wqk_v[:, :, P * f : P * (f + 1)])
                        ps = qkps.tile([P, 512], F32, tag="qk")
                        for c in range(DC):
                            nc.tensor.matmul(
                                ps, wqk_f[:, c, :], xs[:, c, :],
                                start=(c == 0), stop=(c == DC - 1),
                            )
                            yield
                        if f < 4:
                            nc.vector.tensor_copy(qk_q[:, f, jn % 2, :], ps)
                        else:
                            nc.vector.tensor_copy(qk_k[:, f - 4, 512 * jn : 512 * (jn + 1)], ps)
                    for tt in range(4):
                        psv = qkps.tile([P, 512], F32, tag="qk")
                        for c in range(DC):
                            nc.tensor.matmul(
                                psv, xs[:, c, P * tt : P * (tt + 1)], wv_s[:, c, :],
                                start=(c == 0), stop=(c == DC - 1),
                            )
                            yield
                        nc.vector.tensor_copy(
                            vt[:, 4 * jn + tt, :, 0:DH],
                            psv.rearrange("p (h d) -> p h d", h=HPC),
                        )
                if j >= 1:
                    jp = j - 1
                    r = jp % 2
                    for lt in range(4):
                        tt = 4 * jp + lt
                        for e in range(2):
                            pp = qkps.tile([P, 512], F32, tag="qk")
                            for c in range(HPC * DH // P):
                                nc.tensor.matmul(
                                    pp,
                                    yt[:, c, r, P * lt : P * (lt + 1)],
                                    wp[:, c, 512 * e : 512 * (e + 1)],
                                    start=(c == 0),
                                    stop=(c == HPC * DH // P - 1),
                                )
                                yield
                            ob = outsb.tile([P, 512], F32, tag="ob")
                            nc.vector.tensor_copy(ob, pp)
                            nc.sync.dma_start(
                                out_d[P * tt : P * (tt + 1), 512 * e : 512 * (e + 1)], ob
                            )

            def emit_proj_direct(j):
                r = j % 2
                for lt in range(4):
                    tt = 4 * j + lt
                    for e in range(2):
                        pp = qkps.tile([P, 512], F32, tag="qk")
                        for c in range(HPC * DH // P):
                            nc.tensor.matmul(
                                pp,
                                yt[:, c, r, P * lt : P * (lt + 1)],
                                wp[:, c, 512 * e : 512 * (e + 1)],
                                start=(c == 0),
                                stop=(c == HPC * DH // P - 1),
                            )
                        ob = outsb.tile([P, 512], F32, tag="ob")
                        nc.vector.tensor_copy(ob, pp)
                        nc.sync.dma_start(
                            out_d[P * tt : P * (tt + 1), 512 * e : 512 * (e + 1)], ob
                        )

            # per diagonal position p: column offset the tile is computed from
            C0 = (0, 128, 256, 256)

            def attn_head(j, h, fill):
                r = j % 2
                kmax = 4 * j + 4
                hoff = (h % 2) * 64
                qf, kf = h // 2, h // 2
                pv = pvps.tile([P, 512], F32, tag="pv")

                def c0_of(i):
                    return C0[i - 4 * j] if i >= 4 * j else 0

                p_tiles = {}

                def emit_s(i):
                    c0 = c0_of(i)
                    s_ps = sps.tile([P, 512], F32, tag="s")
                    nc.tensor.matmul(
                        s_ps[:, c0:512],
                        qk_k[hoff : hoff + 64, kf, P * i : P * (i + 1)],
                        qk_q[hoff : hoff + 64, qf, r, c0:512],
                        start=True,
                        stop=True,
                    )
                    p_sb = ppool.tile([P, 512], F32R, tag="p")
                    nc.scalar.activation(p_sb[:, c0:512], s_ps[:, c0:512], EXP, scale=1.0 / 8.0)
                    if i >= 4 * j:
                        p = i - 4 * j
                        if p == 3:
                            nc.vector.tensor_mul(
                                p_sb[:, 256:512], p_sb[:, 256:512], masks[:, 128:384]
                            )
                        else:
                            nc.vector.tensor_mul(
                                p_sb[:, P * p : P * (p + 1)],
                                p_sb[:, P * p : P * (p + 1)],
                                masks[:, 0:128],
                            )
                    p_tiles[i] = p_sb

                def emit_pv(i):
                    c0 = c0_of(i)
                    nc.tensor.matmul(
                        pv[0:65, c0:512],
                        vt[:, i, h, :],
                        p_tiles.pop(i)[:, c0:512],
                        start=(i == 0),
                        stop=(i == kmax - 1),
                    )

                SKEW = 3
                for i in range(kmax + SKEW):
                    if i < kmax:
                        emit_s(i)
                    if i >= SKEW:
                        emit_pv(i - SKEW)
                    fill()
                # stash l and unnormalized y^T; batched reciprocal at slab end
                nc.vector.tensor_copy(r_all[64:65, :], pv[64:65, :])
                nc.sync.dma_start(lg[h : h + 1, :], r_all[64:65, :])
                if h % 2 == 0:
                    nc.vector.tensor_copy(yt[0:64, qf, r, :], pv[0:64, :])
                else:
                    ytmp = tails.tile([64, 512], F32R, tag="ytmp")
                    nc.vector.tensor_copy(ytmp, pv[0:64, :])
                    nc.sync.dma_start(yt[64:128, qf, r, :], ytmp)

            def slab_tail(j):
                r = j % 2
                nc.vector.reciprocal(lg[0:HPC, :], lg[0:HPC, :])
                for h in range(HPC):
                    r0 = tails.tile([1, 512], F32, tag="r0")
                    nc.sync.dma_start(r0, lg[h : h + 1, :])
                    rb = tails.tile([P, 512], F32, tag="rb")
                    nc.gpsimd.partition_broadcast(rb, r0, channels=P)
                    qf = h // 2
                    if h % 2 == 0:
                        nc.vector.tensor_mul(yt[0:64, qf, r, :], yt[0:64, qf, r, :], rb[0:64, :])
                    else:
                        nc.vector.tensor_mul(
                            yt[64:128, qf, r, :], yt[64:128, qf, r, :], rb[64:128, :]
                        )

            # ---- pipelined emission ----
            # prologue: slab 0's QKV, emitted directly (PE has nothing else yet)
            xs0 = xpool.tile([P, DC, 512], F32R, tag="xs")
            for c in range(DC):
                nc.sync.dma_start(xs0[:, c, :], xt_v[:, c, 0:512])
            for f in range(8):
                wqk_f = wqkfpool.tile([P, DC, P], F32R, tag="wqkf")
                nc.sync.dma_start(wqk_f, wqk_v[:, :, P * f : P * (f + 1)])
                ps = qkps.tile([P, 512], F32, tag="qk")
                for c in range(DC):
                    nc.tensor.matmul(
                        ps, wqk_f[:, c, :], xs0[:, c, :], start=(c == 0), stop=(c == DC - 1)
                    )
                if f < 4:
                    nc.vector.tensor_copy(qk_q[:, f, 0, :], ps)
                else:
                    nc.vector.tensor_copy(qk_k[:, f - 4, 0:512], ps)
            for tt in range(4):
                psv = qkps.tile([P, 512], F32, tag="qk")
                for c in range(DC):
                    nc.tensor.matmul(
                        psv, xs0[:, c, P * tt : P * (tt + 1)], wv_s[:, c, :],
                        start=(c == 0), stop=(c == DC - 1),
                    )
                nc.vector.tensor_copy(
                    vt[:, tt, :, 0:DH], psv.rearrange("p (h d) -> p h d", h=HPC)
                )

            FILLER_STEPS = {0: 96, 1: 128, 2: 128, 3: 32}
            for j in range(NSLAB):
                gen = filler_gen(j)
                steps_left = FILLER_STEPS[j]
                iters_left = HPC * (4 * j + 4 + 3)  # k-iters incl. skew drain

                def fill():
                    nonlocal steps_left, iters_left
                    want = (steps_left + iters_left - 1) // iters_left if iters_left > 0 else steps_left
                    for _ in range(want):
                        if next(gen, "END") == "END":
                            steps_left = 0
                            break
                        steps_left -= 1
                    iters_left -= 1

                for h in range(HPC):
                    attn_head(j, h, fill)
                while next(gen, "END") != "END":
                    pass
                slab_tail(j)
            emit_proj_direct(NSLAB - 1)
        lp.__exit__(None, None, None)
    nc.compile()
    return nc


def _host_masks():
    m = np.zeros((P, 384), dtype=np.float32)
    for kl in range(P):
        m[kl, 0:128] = (np.arange(128) >= kl).astype(np.float32)       # triangle
        m[kl, 128:384] = (np.arange(256, 512) >= kl + 384).astype(np.float32)  # p=3 tail
    return m


def kernel(x, w_attn, w_proj):
    global _cached_nc, LAST_EXEC_NS
    x = np.asarray(x, dtype=np.float32)
    w_attn = np.asarray(w_attn, dtype=np.float32)
    w_proj = np.asarray(w_proj, dtype=np.float32)

    if _cached_nc is None:
        _cached_nc = _build_program()
    nc = _cached_nc

    masks = _host_masks()
    in_maps = []
    for c in range(N_CORES):
        b, hg = c // 2, (c % 2) * HPC
        w_q = w_attn[hg * DH : hg * DH + HPC * DH, :]
        w_k = w_attn[D + hg * DH : D + hg * DH + HPC * DH, :]
        w_v = w_attn[2 * D + hg * DH : 2 * D + hg * DH + HPC * DH, :]
        in_maps.append(
            {
                "xt": np.ascontiguousarray(x[b].T),
                "wqk": np.ascontiguousarray(np.concatenate([w_q, w_k], axis=0).T),
                "wv": np.ascontiguousarray(w_v.T),
                "wp": np.ascontiguousarray(w_proj[:, hg * DH : hg * DH + HPC * DH].T),
                "masks": masks,
            }
        )

    res = run_bass_kernel_spmd(nc, in_maps, list(range(N_CORES)))
    LAST_EXEC_NS = res.exec_time_ns
    y = np.empty((B, T, D), dtype=np.float32)
    for b in range(B):
        y[b] = res.results[2 * b]["out"] + res.results[2 * b + 1]["out"]
    return y
